# revision 23
# baseline (speedup 1.0000x reference)
"""DepthCueExtractor Trainium2 kernel.

Computes out[b,u,y,x,f] = mean_c(lfi[b,u,y,x,c]) * (S[b,y,f] / max_w S[b,w,f])
where S[b,w,f] = sum_h f_maps[b,h,w,f]  (the 1/H of the mean cancels in the
ratio; the 1/C of the channel mean is folded into the mask).

Sharding: the x (W) axis is split across 8 cores (8 columns each). Every core
receives the full f_maps (the mask needs all h and all w) plus its lfi
x-slice, and writes its out x-slice. All cores run one identical program.

Per-core device plan:
  Stage 0 (mask): load f_maps as two [128, 2048] tiles (partitions = (b,h)
    rows), PE-matmul against 0/1 selection matrices to sum over h, landing
    S replicated over all 128 partitions in the (u,b)-interleaved layout
    (partition p holds S[p % 4]).  Then reduce_max over w, reciprocal,
    and a stride-0-broadcast multiply produce mask[p, (y,f)] in SBUF.
  Stage 1 (main): loop over u-tiles with partitions p = u*4 + b.  lfi loads
    and out stores are fully contiguous per partition (8 KB / 64 KB runs).
    reduce_sum over c gives T[p, (y,j)]; one tensor_tensor multiply with
    stride-0 broadcast APs produces out[p, (y,j,f)] per y-chunk.
"""

import numpy as np

B, U, H, W, C, F = 4, 81, 64, 64, 4, 32
NCORES = 8
XS = W // NCORES  # 8 x-columns per core
YS = 16  # y-chunk per output subtile

_NC_CACHE = {}


def _build_nc(repeat=1, dyn=0, variant="full", ys=None):
    key = (repeat, dyn, variant, ys)
    if key in _NC_CACHE:
        return _NC_CACHE[key]

    from contextlib import ExitStack

    import concourse.bacc as bacc
    import concourse.bass as bass
    import concourse.mybir as mybir
    import concourse.tile as tile

    dt = mybir.dt.float32
    P = 128

    nc = bacc.Bacc("TRN2")
    lfi = nc.dram_tensor("lfi_s", [B, U, H, XS, C], dt, kind="ExternalInput")
    fm = nc.dram_tensor("fm", [B, H, W, F], dt, kind="ExternalInput")
    out = nc.dram_tensor("out_s", [B, U, H, XS, F], dt, kind="ExternalOutput")

    # DRAM strides in elements
    LFI_U, LFI_B = H * XS * C, U * H * XS * C  # 2048, 165888
    OUT_U, OUT_B = H * XS * F, U * H * XS * F  # 16384, 1327104
    OUT_Y = XS * F  # 256

    with tile.TileContext(nc) as tc:
        with ExitStack() as ctx:
            const = ctx.enter_context(tc.tile_pool(name="const", bufs=1))
            psum = ctx.enter_context(tc.tile_pool(name="psum", bufs=1, space="PSUM"))
            lpool = ctx.enter_context(tc.tile_pool(name="lpool", bufs=3))
            obufs = 4 if (ys or YS) <= 16 else 3
            opool = ctx.enter_context(tc.tile_pool(name="opool", bufs=obufs))

            if dyn:
                with tc.For_i(0, dyn, 1):
                    _emit_body(nc, tc, bass, mybir, const, psum, lpool, opool,
                               lfi, fm, out, 0, variant, ys)
            else:
                for _rep in range(repeat):
                    _emit_body(nc, tc, bass, mybir, const, psum, lpool, opool,
                               lfi, fm, out, _rep, variant, ys)

    nc.compile()
    _NC_CACHE[key] = nc
    return nc


def _emit_body(nc, tc, bass, mybir, const, psum, lpool, opool, lfi, fm, out, rep,
               variant="full", ys_chunk=None):
    dt = mybir.dt.float32
    P = 128
    LFI_U, LFI_B = H * XS * C, U * H * XS * C
    OUT_U, OUT_B = H * XS * F, U * H * XS * F
    OUT_Y = XS * F
    if ys_chunk is None:
        ys_chunk = YS
    engines = [nc.sync, nc.scalar, nc.gpsimd]
    do_loads = variant in ("full", "load", "loadcompute")
    do_mask = variant in ("full", "loadcompute")
    do_compute = variant in ("full", "loadcompute")
    do_stores = variant in ("full", "store")

    if variant == "empty":
        z = const.tile([P, 8], dt, tag="z")
        nc.vector.memset(z[:, 0:1], float(rep + 1))
        return

    if variant.startswith("storebig"):
        nways = int(variant[8:]) if len(variant) > 8 else 1
        engines = [nc.sync, nc.scalar, nc.gpsimd][:nways]
        Osrc = const.tile([P, H * XS * F], dt, tag="Obig")
        nc.vector.memset(Osrc[:, 0:1], float(rep + 1))
        for i, (u0, nu) in enumerate([(0, 32), (32, 32), (64, 17)]):
            rows = nu * 4
            dst2 = bass.AP(
                tensor=out,
                offset=u0 * OUT_U,
                ap=[[OUT_U, nu], [OUT_B, 4], [1, H * XS * F]],
            )
            engines[i % nways].dma_start(out=dst2, in_=Osrc[:rows])
        return

    if variant.startswith("store"):
        # "store{W}" or "store{W}y{Y}": W-way engine split, y-chunk Y
        spec = variant[5:]
        if "y" in spec:
            wpart, ypart = spec.split("y")
            nways, ys_chunk = int(wpart), int(ypart)
        else:
            nways, ys_chunk = int(spec or 1), YS
        engines = [nc.sync, nc.scalar, nc.gpsimd][:nways]
        Osrc = const.tile([P, ys_chunk * XS * F], dt, tag="Osrc")
        nc.vector.memset(Osrc[:, 0:1], float(rep + 1))
        i = 0
        for u0, nu in [(0, 32), (32, 32), (64, 17)]:
            rows = nu * 4
            for ys in range(0, H, ys_chunk):
                dst2 = bass.AP(
                    tensor=out,
                    offset=u0 * OUT_U + ys * OUT_Y,
                    ap=[[OUT_U, nu], [OUT_B, 4], [1, ys_chunk * XS * F]],
                )
                engines[i % nways].dma_start(out=dst2, in_=Osrc[:rows])
                i += 1
        return

    if variant == "load":
        fm_flat = fm[:].rearrange("b h w f -> (b h) (w f)")
        for t in range(2):
            ft = const.tile([P, W * F], dt, tag=f"fsb{t}")
            nc.sync.dma_start(out=ft[:], in_=fm_flat[t * P : (t + 1) * P, :])
        for u0, nu in [(0, 32), (32, 32), (64, 17)]:
            rows = nu * 4
            L = lpool.tile([P, H * XS * C], dt, tag="L")
            src = bass.AP(
                tensor=lfi,
                offset=u0 * LFI_U,
                ap=[[LFI_U, nu], [LFI_B, 4], [1, H * XS * C]],
            )
            nc.sync.dma_start(out=L[:rows], in_=src)
            # consume so the loop body isn't empty
            Tt = lpool.tile([P, 1], dt, tag="T1")
            nc.vector.tensor_copy(Tt[:rows], L[:rows, 0:1])
        return

    if True:
        if True:
            # ---- Stage 0: mask ----
            fm_flat = fm[:].rearrange("b h w f -> (b h) (w f)")  # [256, 2048]
            f_sb = []
            li = 0
            for t in range(2):
                ft = const.tile([P, W * F], dt, tag=f"fsb{t}")
                # chunked so the PE matmuls can start before the full tile lands
                for cnk in range(4):
                    engines[li % 3].dma_start(
                        out=ft[:, cnk * 512 : (cnk + 1) * 512],
                        in_=fm_flat[t * P : (t + 1) * P, cnk * 512 : (cnk + 1) * 512],
                    )
                    li += 1
                f_sb.append(ft)

            # sel_t[r, m] = 1 iff m % 4 == 2*t + r//64  (b of fm row == b of
            # output partition m in the (u,b) interleave)
            sel = []
            for t in range(2):
                st = const.tile([P, P], dt, tag=f"sel{t}")
                nc.vector.memset(st[:], 0.0)
                for h2 in range(2):
                    bb = 2 * t + h2
                    view = st[64 * h2 : 64 * (h2 + 1), :].rearrange(
                        "p (m q) -> p m q", q=4
                    )[:, :, bb : bb + 1]
                    nc.vector.memset(view, 1.0)
                sel.append(st)

            psum_S = psum.tile([P, W * F], dt)  # S[p%4, (w,f)] replicated
            for cnk in range(4):
                for t in range(2):
                    nc.tensor.matmul(
                        psum_S[:, cnk * 512 : (cnk + 1) * 512],
                        sel[t][:],
                        f_sb[t][:, cnk * 512 : (cnk + 1) * 512],
                        start=(t == 0),
                        stop=(t == 1),
                    )

            # m[p, f] = max_w S ; r = 0.25 / m
            m_sb = const.tile([P, F], dt)
            nc.vector.reduce_max(
                m_sb[:],
                psum_S[:].rearrange("p (y f) -> p f y", f=F),
                axis=mybir.AxisListType.X,
            )
            r_sb = const.tile([P, F], dt)
            nc.vector.reciprocal(r_sb[:], m_sb[:])
            nc.vector.tensor_scalar_mul(r_sb[:], r_sb[:], 1.0 / C)

            # mask[p, (y,f)] = S * r  (r broadcast along y via stride-0)
            mask_sb = const.tile([P, W * F], dt)
            r_ap = r_sb[:]
            r_bcast = bass.AP(
                tensor=r_ap.tensor,
                offset=r_ap.offset,
                ap=[r_ap.ap[0], [0, W], r_ap.ap[1]],
            )
            nc.vector.tensor_tensor(
                out=mask_sb[:].rearrange("p (y f) -> p y f", f=F),
                in0=psum_S[:].rearrange("p (y f) -> p y f", f=F),
                in1=r_bcast,
                op=mybir.AluOpType.mult,
            )

            # ---- Stage 1: main loop over u-tiles (partitions p = u*4 + b) ----
            u_tiles = [(0, 32), (32, 32), (64, 17)]
            st_i = 0
            for ti, (u0, nu) in enumerate(u_tiles):
                rows = nu * 4
                L = lpool.tile([P, H * XS * C], dt, tag="L")
                src = bass.AP(
                    tensor=lfi,
                    offset=u0 * LFI_U,
                    ap=[[LFI_U, nu], [LFI_B, 4], [1, H * XS * C]],
                )
                engines[ti % 3].dma_start(out=L[:rows], in_=src)

                T = lpool.tile([P, H * XS], dt, tag="T")
                for ys in range(0, H, ys_chunk):
                    nc.vector.reduce_sum(
                        T[:rows, ys * XS : (ys + ys_chunk) * XS],
                        L[:rows, ys * XS * C : (ys + ys_chunk) * XS * C].rearrange(
                            "p (yj c) -> p yj c", c=C
                        ),
                        axis=mybir.AxisListType.X,
                    )

                for ys in range(0, H, ys_chunk):
                    O = opool.tile([P, ys_chunk * XS * F], dt, tag="O")
                    # in0: T[p, y, j] with f broadcast (stride 0)
                    t_ap = T[:rows, ys * XS : (ys + ys_chunk) * XS].rearrange(
                        "p (y j) -> p y j", j=XS
                    )
                    t_bcast = bass.AP(
                        tensor=t_ap.tensor,
                        offset=t_ap.offset,
                        ap=list(t_ap.ap) + [[0, F]],
                    )
                    # in1: mask[p, y, f] with j broadcast (stride 0)
                    m_ap = mask_sb[:rows, ys * F : (ys + ys_chunk) * F].rearrange(
                        "p (y f) -> p y f", f=F
                    )
                    m_bcast = bass.AP(
                        tensor=m_ap.tensor,
                        offset=m_ap.offset,
                        ap=[m_ap.ap[0], m_ap.ap[1], [0, XS], m_ap.ap[2]],
                    )
                    nc.vector.tensor_tensor(
                        out=O[:rows].rearrange("p (y j f) -> p y j f", j=XS, f=F),
                        in0=t_bcast,
                        in1=m_bcast,
                        op=mybir.AluOpType.mult,
                    )
                    if do_stores:
                        dst2 = bass.AP(
                            tensor=out,
                            offset=u0 * OUT_U + ys * OUT_Y,
                            ap=[[OUT_U, nu], [OUT_B, 4], [1, ys_chunk * XS * F]],
                        )
                        engines[st_i % 3].dma_start(out=dst2, in_=O[:rows])
                        st_i += 1


def kernel(lfi, f_maps):
    from concourse.bass_utils import run_bass_kernel_spmd

    nc = _build_nc()
    fm = np.ascontiguousarray(f_maps, dtype=np.float32)
    in_maps = []
    for k in range(NCORES):
        sl = np.ascontiguousarray(
            lfi[:, :, :, k * XS : (k + 1) * XS, :], dtype=np.float32
        )
        in_maps.append({"lfi_s": sl, "fm": fm})
    res = run_bass_kernel_spmd(nc, in_maps, core_ids=list(range(NCORES)))
    outs = [r["out_s"] for r in res.results]
    return np.concatenate(outs, axis=3)
